# revision 13
# baseline (speedup 1.0000x reference)
"""Multi-head attention (B=2, S=2048, D=1024, H=16, causal) on 8 TRN2 NeuronCores.

Sharding: core c handles batch c//4 and heads [4*(c%4), 4*(c%4)+4) —
data-parallel over batch x tensor-parallel over heads, Megatron-style:
QKV projection weights are column-split (each core computes only its own
heads' features), the output projection is row-split (each core emits a
full-width partial that the host sums).

Per-core device kernel (bf16 matmul operands, fp32 accumulation):
  - Q,K projected feature-major (QT/KT = W_local @ x^T, shape (256, 2048))
    so the scores matmul needs no on-device transposes.
  - V projected in natural (seq, feat) layout with a fused ones-column so
    a single PV matmul produces both attn@V and the softmax denominator.
  - scores^T per (head, q-chunk, key-chunk): K^T-chunk stationary, Q moving.
  - softmax without max-subtraction (scores ~ N(0,1); exp is accurate
    enough), causal handled by skipping upper-triangle key chunks and
    affine_select-masking the 4 diagonal chunk patterns.
  - normalization: Z row broadcast across partitions with a K=1 fp32
    matmul, 1/Z on VectorE (DVE reciprocal — ScalarE Ln/Exp would thrash
    ACT table sets; custom-DVE/partition_broadcast mis-execute on HW).
  - O projection contracts the 256 local features against Wo rows; the
    partial output is written feature-major (1024, 2048) fp32 and the
    host transposes/sums partials and adds bo.
"""

import numpy as np
import ml_dtypes

import concourse.bacc as bacc
import concourse.mybir as mybir
import concourse.tile as tile
from concourse.bass_utils import run_bass_kernel_spmd

B, S, D, H = 2, 2048, 1024, 16
DK = D // H           # 64, head dim
DL = 256              # local (per-core) projected features = 4 heads
NHL = 4               # heads per core
NQ = 4                # q-chunks of 512
F32 = mybir.dt.float32
BF16 = mybir.dt.bfloat16
NPBF16 = ml_dtypes.bfloat16


def _emit(tc, io):
    nc = tc.nc
    qt, kt, vt = io["qt"], io["kt"], io["vt"]          # (1024, 2048) bf16
    wqt, wkt, wvt = io["wqt"], io["wkt"], io["wvt"]    # (1024, 256) bf16
    wot = io["wot"]                                    # (256, 1024) bf16
    bqc, bkc = io["bqc"], io["bkc"]                    # (128, 2) f32
    bvr = io["bvr"]                                    # (1, 256) bf16
    outp = io["outp"]                                  # (1024, 2048) f32
    EXP = mybir.ActivationFunctionType.Exp

    with (
        tc.tile_pool(name="const", bufs=1) as cw,
        tc.tile_pool(name="io", bufs=10) as iop,
        tc.tile_pool(name="big", bufs=1) as big,
        tc.tile_pool(name="work", bufs=3) as wk,
        tc.tile_pool(name="psA", bufs=2, space="PSUM") as psA,
        tc.tile_pool(name="psB", bufs=2, space="PSUM") as psB,
        tc.tile_pool(name="psC", bufs=2, space="PSUM") as psC,
    ):
        ones_sb = cw.tile([128, 128], BF16)
        nc.gpsimd.memset(ones_sb[:], 1.0)
        onesf = cw.tile([65, 64], F32)   # fp32 ones, row 64 used as K=1 lhsT
        nc.gpsimd.memset(onesf[:], 1.0)
        bq_sb = cw.tile([128, 2], F32)
        nc.sync.dma_start(bq_sb[:], bqc[:, :])
        bk_sb = cw.tile([128, 2], F32)
        nc.sync.dma_start(bk_sb[:], bkc[:, :])
        bv_sb = cw.tile([1, 256], BF16)
        nc.sync.dma_start(bv_sb[:], bvr[:, :])

        wq_sb = cw.tile([128, 8, 256], BF16)
        nc.sync.dma_start(wq_sb[:], wqt[:, :].rearrange("(k p) m -> p k m", p=128))
        wk_sb = cw.tile([128, 8, 256], BF16)
        nc.sync.dma_start(wk_sb[:], wkt[:, :].rearrange("(k p) m -> p k m", p=128))
        wv_sb = cw.tile([128, 8, 256], BF16)
        nc.sync.dma_start(wv_sb[:], wvt[:, :].rearrange("(k p) m -> p k m", p=128))
        wo_sb = cw.tile([128, 2, 1024], BF16)
        nc.sync.dma_start(wo_sb[:], wot[:, :].rearrange("(c p) m -> p c m", p=128))

        QT = big.tile([128, 2, S], BF16)   # [feat%128, feat//128, seq]
        KT = big.tile([128, 2, S], BF16)
        VA = big.tile([128, NHL, 16, 65], BF16)  # [key%128, head, key//128, dk|1]
        ON = big.tile([128, 2, S], BF16)   # normalized attn out, feature-major
        nc.gpsimd.memset(VA[:, :, :, 64:65], 1.0)

        # ---- Q/K projections, feature-major: dst[:, m, n] = W_local @ x^T ----
        # inputs stream as full 512KB rows (4KB/partition descriptors — small
        # per-partition runs were tanking HW-DGE efficiency)
        for xd, w_sb, b_sb, dst in (
            (qt, wq_sb, bq_sb, QT),
            (kt, wk_sb, bk_sb, KT),
        ):
            rows = []
            for k in range(8):
                r = iop.tile([128, S], BF16, tag="xrow", name=f"xr{k}")
                nc.sync.dma_start(r[:], xd[k * 128 : (k + 1) * 128, :])
                rows.append(r)
            for n in range(NQ):
                pm = [
                    psA.tile([128, 512], F32, tag="proj", name=f"pm{m}")
                    for m in range(2)
                ]
                for k in range(8):
                    for m in range(2):
                        nc.tensor.matmul(
                            pm[m][:],
                            w_sb[:, k, m * 128 : (m + 1) * 128],
                            rows[k][:, n * 512 : (n + 1) * 512],
                            start=(k == 0),
                            stop=(k == 7),
                        )
                for m in range(2):
                    nc.vector.tensor_scalar_add(
                        dst[:, m, n * 512 : (n + 1) * 512], pm[m][:], b_sb[:, m : m + 1]
                    )

        # ---- V projection, natural layout, bias via K=1 ones matmul ----
        vrows = []
        for k in range(8):
            r = iop.tile([128, S], BF16, tag="xrow", name=f"vr{k}")
            nc.sync.dma_start(r[:], vt[k * 128 : (k + 1) * 128, :])
            vrows.append(r)
        for sp in range(8):
            pvps = psA.tile([128, 512], F32, tag="proj")
            for k in range(8):
                for half in range(2):
                    s = sp * 256 + half * 128
                    nc.tensor.matmul(
                        pvps[:, half * 256 : (half + 1) * 256],
                        vrows[k][:, s : s + 128],
                        wv_sb[:, k, :],
                        start=(k == 0 and half == 0),
                        stop=False,
                    )
            for half in range(2):
                nc.tensor.matmul(
                    pvps[:, half * 256 : (half + 1) * 256],
                    ones_sb[0:1, 0:128],
                    bv_sb[:],
                    start=False,
                    stop=(half == 1),
                )
            for half in range(2):
                s = sp * 2 + half
                nc.vector.tensor_copy(
                    VA[:, :, s, 0:64],
                    pvps[:, half * 256 : (half + 1) * 256].rearrange(
                        "p (h d) -> p h d", d=64
                    ),
                )

        # ---- attention + output projection, q-chunk-major for overlap ----
        # O-proj for chunk j is emitted after attention chunk j+1 so the PE
        # never waits on the normalize chain (reciprocal on DVE) of chunk j.
        def emit_oproj(j):
            for m in range(8):
                po = psA.tile([128, 512], F32, tag="proj", name="po")
                for c in range(2):
                    nc.tensor.matmul(
                        po[:],
                        wo_sb[:, c, m * 128 : (m + 1) * 128],
                        ON[:, c, j * 512 : (j + 1) * 512],
                        start=(c == 0),
                        stop=(c == 1),
                    )
                ot = wk.tile([128, 512], F32, tag="ot", name="ot")
                nc.vector.tensor_copy(ot[:], po[:])
                nc.sync.dma_start(
                    outp[m * 128 : (m + 1) * 128, j * 512 : (j + 1) * 512], ot[:]
                )

        # chunks processed j descending (longest key runs right after the
        # projections keep the PE dense); two heads interleaved per stream so
        # the PE always has independent matmuls while exp/normalize chains run
        jorder = list(range(NQ - 1, -1, -1))
        for idx, j in enumerate(jorder):
            nkc = 4 * (j + 1)
            for hp in range(2):
                pvs = [
                    psC.tile([65, 512], F32, tag="pv", name=f"pv{i}")
                    for i in range(2)
                ]
                for kp in range(nkc // 2):
                    for hi in range(2):
                        h = 2 * hp + hi
                        p0 = 64 * hi
                        # exp batched over 2 key-chunks (one 2-bank PSUM
                        # tile) to amortize ACTIVATE's ~350ns fixed cost
                        scp = psB.tile([128, 1024], F32, tag="sc", name="scp")
                        for half in range(2):
                            kc = 2 * kp + half
                            nc.tensor.matmul(
                                scp[:, half * 512 : (half + 1) * 512],
                                KT[p0 : p0 + 64, hp, kc * 128 : (kc + 1) * 128],
                                QT[p0 : p0 + 64, hp, j * 512 : (j + 1) * 512],
                                start=True,
                                stop=True,
                            )
                        et = wk.tile([128, 1024], BF16, tag="exp", name="et")
                        nc.scalar.activation(
                            et[:], scp[:], EXP, scale=float(DK) ** -0.5
                        )
                        for half in range(2):
                            kc = 2 * kp + half
                            if kc >= 4 * j:
                                t = kc - 4 * j
                                nc.gpsimd.affine_select(
                                    out=et[:, half * 512 : (half + 1) * 512],
                                    in_=et[:, half * 512 : (half + 1) * 512],
                                    compare_op=mybir.AluOpType.is_ge,
                                    fill=0.0,
                                    base=-128 * t,
                                    pattern=[[1, 512]],
                                    channel_multiplier=-1,
                                )
                        for half in range(2):
                            kc = 2 * kp + half
                            nc.tensor.matmul(
                                pvs[hi][:],
                                VA[:, h, kc, :],
                                et[:, half * 512 : (half + 1) * 512],
                                start=(kc == 0),
                                stop=(kc == nkc - 1),
                            )
                # copy each PV accumulator to SBUF right away so its PSUM
                # slot frees fast (holding it through the reciprocal chain
                # stalled the PE and re-throttled the HAM clock gate), then
                # normalize from SBUF: broadcast Z across 64 partitions with
                # a K=1 fp32 matmul, 1/Z on DVE, scale into ON.
                for hi in range(2):
                    u = wk.tile([65, 512], F32, tag="u", name="u")
                    nc.vector.tensor_copy(u[:], pvs[hi][:])
                    rbps = psA.tile([64, 512], F32, tag="proj", name="rbps")
                    nc.tensor.matmul(
                        rbps[:], onesf[64:65, :], u[64:65, :], start=True, stop=True
                    )
                    rb = wk.tile([64, 512], F32, tag="rb", name="rb")
                    nc.vector.reciprocal(rb[:], rbps[:])
                    if hi == 0:
                        nc.vector.tensor_mul(
                            ON[0:64, hp, j * 512 : (j + 1) * 512], u[0:64, :], rb[:]
                        )
                    else:
                        nt = wk.tile([64, 512], BF16, tag="nt", name="nt")
                        nc.vector.tensor_mul(nt[:], u[0:64, :], rb[:])
                        nc.sync.dma_start(
                            ON[64:128, hp, j * 512 : (j + 1) * 512], nt[:]
                        )
            if idx > 0:
                emit_oproj(jorder[idx - 1])
        emit_oproj(jorder[-1])


def build_nc():
    nc = bacc.Bacc("TRN2", target_bir_lowering=False, debug=False, num_devices=8)
    io = {}
    for name, shape, dt in (
        ("qt", (D, S), BF16),
        ("kt", (D, S), BF16),
        ("vt", (D, S), BF16),
        ("wqt", (D, DL), BF16),
        ("wkt", (D, DL), BF16),
        ("wvt", (D, DL), BF16),
        ("wot", (DL, D), BF16),
        ("bqc", (128, 2), F32),
        ("bkc", (128, 2), F32),
        ("bvr", (1, DL), BF16),
    ):
        io[name] = nc.dram_tensor(name, shape, dt, kind="ExternalInput")
    io["outp"] = nc.dram_tensor("outp", (D, S), F32, kind="ExternalOutput")
    with tile.TileContext(nc) as tc:
        _emit(tc, io)
    nc.compile()
    return nc


_NC = None


def _get_nc():
    global _NC
    if _NC is None:
        _NC = build_nc()
    return _NC


def make_in_maps(q, k, v, Wq, bq, Wk, bk, Wv, bv, Wo):
    def cb(x):  # contiguous bf16
        return np.ascontiguousarray(x).astype(NPBF16)

    cf = np.ascontiguousarray
    in_maps = []
    for core in range(8):
        b, g = divmod(core, 4)
        sl = slice(DL * g, DL * (g + 1))
        in_maps.append(
            {
                "qt": cb(q[b].T),
                "kt": cb(k[b].T),
                "vt": cb(v[b].T),
                "wqt": cb(Wq[sl, :].T),
                "wkt": cb(Wk[sl, :].T),
                "wvt": cb(Wv[sl, :].T),
                "wot": cb(Wo[:, sl].T),
                "bqc": cf(bq[sl].reshape(2, 128).T),
                "bkc": cf(bk[sl].reshape(2, 128).T),
                "bvr": cb(bv[sl].reshape(1, DL)),
            }
        )
    return in_maps


def gather_output(results, bo):
    out = np.empty((B, S, D), np.float32)
    for b in range(B):
        acc = results[4 * b]["outp"].astype(np.float32)
        for g in range(1, 4):
            acc = acc + results[4 * b + g]["outp"]
        out[b] = acc.T + bo
    return out


def _np_fallback(q, k, v, mask, Wq, bq, Wk, bk, Wv, bv, Wo, bo):
    # generic-mask reference path; only used if the mask is not causal
    out = np.empty((B, S, D), np.float32)
    m = np.broadcast_to(mask, (B, 1, S, S))
    for b in range(B):
        Q = (q[b] @ Wq.T + bq).reshape(S, H, DK).transpose(1, 0, 2)
        K = (k[b] @ Wk.T + bk).reshape(S, H, DK).transpose(1, 0, 2)
        V = (v[b] @ Wv.T + bv).reshape(S, H, DK).transpose(1, 0, 2)
        o = np.empty((H, S, DK), np.float32)
        for hh in range(H):
            s = (Q[hh] @ K[hh].T) * (DK**-0.5)
            s = np.where(m[b, 0] == 0, -np.inf, s)
            s = s - s.max(axis=-1, keepdims=True)
            e = np.exp(s)
            o[hh] = (e / e.sum(axis=-1, keepdims=True)) @ V[hh]
        out[b] = o.transpose(1, 0, 2).reshape(S, D) @ Wo.T + bo
    return out


def kernel(q, k, v, mask, Wq, bq, Wk, bk, Wv, bv, Wo, bo):
    f32 = np.float32
    q, k, v = (np.asarray(x, f32) for x in (q, k, v))
    Wq, bq, Wk, bk = (np.asarray(x, f32) for x in (Wq, bq, Wk, bk))
    Wv, bv, Wo, bo = (np.asarray(x, f32) for x in (Wv, bv, Wo, bo))
    mask = np.asarray(mask)

    if not np.array_equal(
        np.broadcast_to(mask, (1, 1, S, S))[0, 0] != 0,
        np.tril(np.ones((S, S), bool)),
    ):
        return _np_fallback(q, k, v, mask, Wq, bq, Wk, bk, Wv, bv, Wo, bo)

    nc = _get_nc()
    in_maps = make_in_maps(q, k, v, Wq, bq, Wk, bk, Wv, bv, Wo)
    res = run_bass_kernel_spmd(nc, in_maps, list(range(8)))
    return gather_output(res.results, bo)


# revision 15
# speedup vs baseline: 1.0532x; 1.0532x over previous
"""Multi-head attention (B=2, S=2048, D=1024, H=16, causal) on 8 TRN2 NeuronCores.

Sharding: core c handles batch c//4 and heads [4*(c%4), 4*(c%4)+4) —
data-parallel over batch x tensor-parallel over heads, Megatron-style:
QKV projection weights are column-split (each core computes only its own
heads' features), the output projection is row-split (each core emits a
full-width partial that the host sums).

Per-core device kernel (bf16 matmul operands, fp32 accumulation):
  - Q,K projected feature-major (QT/KT = W_local @ x^T, shape (256, 2048))
    so the scores matmul needs no on-device transposes.
  - V projected in natural (seq, feat) layout with a fused ones-column so
    a single PV matmul produces both attn@V and the softmax denominator.
  - scores^T per (head, q-chunk, key-chunk): K^T-chunk stationary, Q moving.
  - softmax without max-subtraction (scores ~ N(0,1); exp is accurate
    enough), causal handled by skipping upper-triangle key chunks and
    affine_select-masking the 4 diagonal chunk patterns.
  - normalization: Z row broadcast across partitions with a K=1 fp32
    matmul, 1/Z on VectorE (DVE reciprocal — ScalarE Ln/Exp would thrash
    ACT table sets; custom-DVE/partition_broadcast mis-execute on HW).
  - O projection contracts the 256 local features against Wo rows; the
    partial output is written feature-major (1024, 2048) fp32 and the
    host transposes/sums partials and adds bo.
"""

import numpy as np
import ml_dtypes

import concourse.bacc as bacc
import concourse.mybir as mybir
import concourse.tile as tile
from concourse.bass_utils import run_bass_kernel_spmd

B, S, D, H = 2, 2048, 1024, 16
DK = D // H           # 64, head dim
DL = 256              # local (per-core) projected features = 4 heads
NHL = 4               # heads per core
NQ = 4                # q-chunks of 512
F32 = mybir.dt.float32
BF16 = mybir.dt.bfloat16
NPBF16 = ml_dtypes.bfloat16


def _emit(tc, io):
    nc = tc.nc
    qt, kt, vt = io["qt"], io["kt"], io["vt"]          # (1024, 2048) bf16
    wqt, wkt, wvt = io["wqt"], io["wkt"], io["wvt"]    # (1024, 256) bf16
    wot = io["wot"]                                    # (256, 1024) bf16
    bqc, bkc = io["bqc"], io["bkc"]                    # (128, 2) f32
    bvr = io["bvr"]                                    # (1, 256) bf16
    outp = io["outp"]                                  # (1024, 2048) f32
    EXP = mybir.ActivationFunctionType.Exp

    with (
        tc.tile_pool(name="const", bufs=1) as cw,
        tc.tile_pool(name="io", bufs=10) as iop,
        tc.tile_pool(name="big", bufs=1) as big,
        tc.tile_pool(name="work", bufs=3) as wk,
        tc.tile_pool(name="psA", bufs=2, space="PSUM") as psA,
        tc.tile_pool(name="psB", bufs=2, space="PSUM") as psB,
        tc.tile_pool(name="psC", bufs=2, space="PSUM") as psC,
    ):
        ones_sb = cw.tile([128, 128], BF16)
        nc.gpsimd.memset(ones_sb[:], 1.0)
        onesf = cw.tile([65, 64], F32)   # fp32 ones, row 64 used as K=1 lhsT
        nc.gpsimd.memset(onesf[:], 1.0)
        bq_sb = cw.tile([128, 2], F32)
        nc.sync.dma_start(bq_sb[:], bqc[:, :])
        bk_sb = cw.tile([128, 2], F32)
        nc.sync.dma_start(bk_sb[:], bkc[:, :])
        bv_sb = cw.tile([1, 256], BF16)
        nc.sync.dma_start(bv_sb[:], bvr[:, :])

        wq_sb = cw.tile([128, 8, 256], BF16)
        nc.sync.dma_start(wq_sb[:], wqt[:, :].rearrange("(k p) m -> p k m", p=128))
        wk_sb = cw.tile([128, 8, 256], BF16)
        nc.sync.dma_start(wk_sb[:], wkt[:, :].rearrange("(k p) m -> p k m", p=128))
        wv_sb = cw.tile([128, 8, 256], BF16)
        nc.sync.dma_start(wv_sb[:], wvt[:, :].rearrange("(k p) m -> p k m", p=128))
        wo_sb = cw.tile([128, 2, 1024], BF16)
        nc.sync.dma_start(wo_sb[:], wot[:, :].rearrange("(c p) m -> p c m", p=128))

        QT = big.tile([128, 2, S], BF16)   # [feat%128, feat//128, seq]
        KT = big.tile([128, 2, S], BF16)
        VA = big.tile([128, NHL, 16, 65], BF16)  # [key%128, head, key//128, dk|1]
        ON = big.tile([128, 2, S], BF16)   # normalized attn out, feature-major
        nc.gpsimd.memset(VA[:, :, :, 64:65], 1.0)

        # ---- Q/K projections, feature-major: dst[:, m, n] = W_local @ x^T ----
        # inputs stream as full 512KB rows (4KB/partition descriptors — small
        # per-partition runs were tanking HW-DGE efficiency)
        for xd, w_sb, b_sb, dst in (
            (qt, wq_sb, bq_sb, QT),
            (kt, wk_sb, bk_sb, KT),
        ):
            rows = []
            for k in range(8):
                r = iop.tile([128, S], BF16, tag="xrow", name=f"xr{k}")
                nc.sync.dma_start(r[:], xd[k * 128 : (k + 1) * 128, :])
                rows.append(r)
            for n in range(NQ):
                pm = [
                    psA.tile([128, 512], F32, tag="proj", name=f"pm{m}")
                    for m in range(2)
                ]
                for k in range(8):
                    for m in range(2):
                        nc.tensor.matmul(
                            pm[m][:],
                            w_sb[:, k, m * 128 : (m + 1) * 128],
                            rows[k][:, n * 512 : (n + 1) * 512],
                            start=(k == 0),
                            stop=(k == 7),
                        )
                for m in range(2):
                    nc.vector.tensor_scalar_add(
                        dst[:, m, n * 512 : (n + 1) * 512], pm[m][:], b_sb[:, m : m + 1]
                    )

        # ---- V projection, natural layout, bias via K=1 ones matmul ----
        vrows = []
        for k in range(8):
            r = iop.tile([128, S], BF16, tag="xrow", name=f"vr{k}")
            nc.sync.dma_start(r[:], vt[k * 128 : (k + 1) * 128, :])
            vrows.append(r)
        for sp in range(8):
            pvps = psA.tile([128, 512], F32, tag="proj")
            for k in range(8):
                for half in range(2):
                    s = sp * 256 + half * 128
                    nc.tensor.matmul(
                        pvps[:, half * 256 : (half + 1) * 256],
                        vrows[k][:, s : s + 128],
                        wv_sb[:, k, :],
                        start=(k == 0 and half == 0),
                        stop=False,
                    )
            for half in range(2):
                nc.tensor.matmul(
                    pvps[:, half * 256 : (half + 1) * 256],
                    ones_sb[0:1, 0:128],
                    bv_sb[:],
                    start=False,
                    stop=(half == 1),
                )
            for half in range(2):
                s = sp * 2 + half
                nc.vector.tensor_copy(
                    VA[:, :, s, 0:64],
                    pvps[:, half * 256 : (half + 1) * 256].rearrange(
                        "p (h d) -> p h d", d=64
                    ),
                )

        # ---- attention + output projection, q-chunk-major for overlap ----
        # O-proj for chunk j is emitted after attention chunk j+1 so the PE
        # never waits on the normalize chain (reciprocal on DVE) of chunk j.
        def emit_oproj(j):
            for m in range(8):
                po = psA.tile([128, 512], F32, tag="proj", name="po")
                for c in range(2):
                    nc.tensor.matmul(
                        po[:],
                        wo_sb[:, c, m * 128 : (m + 1) * 128],
                        ON[:, c, j * 512 : (j + 1) * 512],
                        start=(c == 0),
                        stop=(c == 1),
                    )
                ot = wk.tile([128, 512], F32, tag="ot", name="ot")
                nc.any.tensor_copy(ot[:], po[:])
                nc.sync.dma_start(
                    outp[m * 128 : (m + 1) * 128, j * 512 : (j + 1) * 512], ot[:]
                )

        for j in range(NQ):
            nkc = 4 * (j + 1)
            us = []
            for h in range(NHL):
                p0 = 64 * (h % 2)
                ch = h // 2
                pvp = psC.tile([65, 512], F32, tag="pv", name="pvp")
                # exp batched over 2 key-chunks (one 2-bank PSUM tile) to
                # amortize the ~350ns fixed cost per ACTIVATE
                for kp in range(nkc // 2):
                    scp = psB.tile([128, 1024], F32, tag="sc", name="scp")
                    for half in range(2):
                        kc = 2 * kp + half
                        nc.tensor.matmul(
                            scp[:, half * 512 : (half + 1) * 512],
                            KT[p0 : p0 + 64, ch, kc * 128 : (kc + 1) * 128],
                            QT[p0 : p0 + 64, ch, j * 512 : (j + 1) * 512],
                            start=True,
                            stop=True,
                        )
                    et = wk.tile([128, 1024], BF16, tag="exp", name="et")
                    nc.scalar.activation(et[:], scp[:], EXP, scale=float(DK) ** -0.5)
                    for half in range(2):
                        kc = 2 * kp + half
                        if kc >= 4 * j:
                            t = kc - 4 * j
                            nc.gpsimd.affine_select(
                                out=et[:, half * 512 : (half + 1) * 512],
                                in_=et[:, half * 512 : (half + 1) * 512],
                                compare_op=mybir.AluOpType.is_ge,
                                fill=0.0,
                                base=-128 * t,
                                pattern=[[1, 512]],
                                channel_multiplier=-1,
                            )
                    for half in range(2):
                        kc = 2 * kp + half
                        nc.tensor.matmul(
                            pvp[:],
                            VA[:, h, kc, :],
                            et[:, half * 512 : (half + 1) * 512],
                            start=(kc == 0),
                            stop=(kc == nkc - 1),
                        )
                # copy the PV accumulator to SBUF immediately — freeing the
                # pv PSUM slot fast keeps the PE from stalling (and HAM from
                # re-throttling the clock); normalization is deferred below.
                u = wk.tile([65, 512], F32, tag="u", name="u", bufs=6)
                nc.any.tensor_copy(u[:], pvp[:])
                us.append(u)
            # O-projection of the previous chunk keeps the PE fed while this
            # chunk's reciprocals run on the DVE
            if j > 0:
                emit_oproj(j - 1)
            # normalize: 1/Z in place on the SBUF Z row (all reciprocals at
            # the chunk boundary so they never delay the PSUM-freeing copies
            # in the DVE queue), broadcast across 64 partitions with a K=1
            # fp32 matmul, scale into ON
            for h in range(NHL):
                u = us[h]
                nc.vector.reciprocal(u[64:65, :], u[64:65, :])
                rbps = psA.tile([64, 512], F32, tag="proj", name="rbps")
                nc.tensor.matmul(
                    rbps[:], onesf[64:65, :], u[64:65, :], start=True, stop=True
                )
                if h % 2 == 0:
                    nc.vector.tensor_mul(
                        ON[0:64, h // 2, j * 512 : (j + 1) * 512], u[0:64, :], rbps[:]
                    )
                else:
                    nt = wk.tile([64, 512], BF16, tag="nt", name="nt")
                    nc.vector.tensor_mul(nt[:], u[0:64, :], rbps[:])
                    nc.sync.dma_start(
                        ON[64:128, h // 2, j * 512 : (j + 1) * 512], nt[:]
                    )
        emit_oproj(NQ - 1)


def build_nc():
    nc = bacc.Bacc("TRN2", target_bir_lowering=False, debug=False, num_devices=8)
    io = {}
    for name, shape, dt in (
        ("qt", (D, S), BF16),
        ("kt", (D, S), BF16),
        ("vt", (D, S), BF16),
        ("wqt", (D, DL), BF16),
        ("wkt", (D, DL), BF16),
        ("wvt", (D, DL), BF16),
        ("wot", (DL, D), BF16),
        ("bqc", (128, 2), F32),
        ("bkc", (128, 2), F32),
        ("bvr", (1, DL), BF16),
    ):
        io[name] = nc.dram_tensor(name, shape, dt, kind="ExternalInput")
    io["outp"] = nc.dram_tensor("outp", (D, S), F32, kind="ExternalOutput")
    with tile.TileContext(nc) as tc:
        _emit(tc, io)
    nc.compile()
    return nc


_NC = None


def _get_nc():
    global _NC
    if _NC is None:
        _NC = build_nc()
    return _NC


def make_in_maps(q, k, v, Wq, bq, Wk, bk, Wv, bv, Wo):
    def cb(x):  # contiguous bf16
        return np.ascontiguousarray(x).astype(NPBF16)

    cf = np.ascontiguousarray
    in_maps = []
    for core in range(8):
        b, g = divmod(core, 4)
        sl = slice(DL * g, DL * (g + 1))
        in_maps.append(
            {
                "qt": cb(q[b].T),
                "kt": cb(k[b].T),
                "vt": cb(v[b].T),
                "wqt": cb(Wq[sl, :].T),
                "wkt": cb(Wk[sl, :].T),
                "wvt": cb(Wv[sl, :].T),
                "wot": cb(Wo[:, sl].T),
                "bqc": cf(bq[sl].reshape(2, 128).T),
                "bkc": cf(bk[sl].reshape(2, 128).T),
                "bvr": cb(bv[sl].reshape(1, DL)),
            }
        )
    return in_maps


def gather_output(results, bo):
    out = np.empty((B, S, D), np.float32)
    for b in range(B):
        acc = results[4 * b]["outp"].astype(np.float32)
        for g in range(1, 4):
            acc = acc + results[4 * b + g]["outp"]
        out[b] = acc.T + bo
    return out


def _np_fallback(q, k, v, mask, Wq, bq, Wk, bk, Wv, bv, Wo, bo):
    # generic-mask reference path; only used if the mask is not causal
    out = np.empty((B, S, D), np.float32)
    m = np.broadcast_to(mask, (B, 1, S, S))
    for b in range(B):
        Q = (q[b] @ Wq.T + bq).reshape(S, H, DK).transpose(1, 0, 2)
        K = (k[b] @ Wk.T + bk).reshape(S, H, DK).transpose(1, 0, 2)
        V = (v[b] @ Wv.T + bv).reshape(S, H, DK).transpose(1, 0, 2)
        o = np.empty((H, S, DK), np.float32)
        for hh in range(H):
            s = (Q[hh] @ K[hh].T) * (DK**-0.5)
            s = np.where(m[b, 0] == 0, -np.inf, s)
            s = s - s.max(axis=-1, keepdims=True)
            e = np.exp(s)
            o[hh] = (e / e.sum(axis=-1, keepdims=True)) @ V[hh]
        out[b] = o.transpose(1, 0, 2).reshape(S, D) @ Wo.T + bo
    return out


def kernel(q, k, v, mask, Wq, bq, Wk, bk, Wv, bv, Wo, bo):
    f32 = np.float32
    q, k, v = (np.asarray(x, f32) for x in (q, k, v))
    Wq, bq, Wk, bk = (np.asarray(x, f32) for x in (Wq, bq, Wk, bk))
    Wv, bv, Wo, bo = (np.asarray(x, f32) for x in (Wv, bv, Wo, bo))
    mask = np.asarray(mask)

    if not np.array_equal(
        np.broadcast_to(mask, (1, 1, S, S))[0, 0] != 0,
        np.tril(np.ones((S, S), bool)),
    ):
        return _np_fallback(q, k, v, mask, Wq, bq, Wk, bk, Wv, bv, Wo, bo)

    nc = _get_nc()
    in_maps = make_in_maps(q, k, v, Wq, bq, Wk, bk, Wv, bv, Wo)
    res = run_bass_kernel_spmd(nc, in_maps, list(range(8)))
    return gather_output(res.results, bo)


# revision 18
# speedup vs baseline: 1.1753x; 1.1159x over previous
"""Multi-head attention (B=2, S=2048, D=1024, H=16, causal) on 8 TRN2 NeuronCores.

Sharding: core c handles batch c//4 and heads [4*(c%4), 4*(c%4)+4) —
data-parallel over batch x tensor-parallel over heads, Megatron-style:
QKV projection weights are column-split (each core computes only its own
heads' features), the output projection is row-split (each core emits a
full-width partial that the host sums).

Per-core device kernel (bf16 matmul operands, fp32 accumulation):
  - Q,K projected feature-major (QT/KT = W_local @ x^T, shape (256, 2048))
    so the scores matmul needs no on-device transposes.
  - V projected in natural (seq, feat) layout with a fused ones-column so
    a single PV matmul produces both attn@V and the softmax denominator.
  - scores^T per (head, q-chunk, key-chunk): K^T-chunk stationary, Q moving.
  - softmax without max-subtraction (scores ~ N(0,1); exp is accurate
    enough), causal handled by skipping upper-triangle key chunks and
    affine_select-masking the 4 diagonal chunk patterns.
  - normalization: Z row broadcast across partitions with a K=1 fp32
    matmul, 1/Z on VectorE (DVE reciprocal — ScalarE Ln/Exp would thrash
    ACT table sets; custom-DVE/partition_broadcast mis-execute on HW).
  - O projection contracts the 256 local features against Wo rows; the
    partial output is written feature-major (1024, 2048) fp32 and the
    host transposes/sums partials and adds bo.
"""

import numpy as np
import ml_dtypes

import concourse.bacc as bacc
import concourse.mybir as mybir
import concourse.tile as tile
from concourse.bass_utils import run_bass_kernel_spmd

B, S, D, H = 2, 2048, 1024, 16
DK = D // H           # 64, head dim
DL = 256              # local (per-core) projected features = 4 heads
NHL = 4               # heads per core
NQ = 4                # q-chunks of 512
F32 = mybir.dt.float32
BF16 = mybir.dt.bfloat16
NPBF16 = ml_dtypes.bfloat16


def _emit(tc, io):
    nc = tc.nc
    qt, kt, vt = io["qt"], io["kt"], io["vt"]          # (1024, 2048) bf16
    wqt, wkt, wvt = io["wqt"], io["wkt"], io["wvt"]    # (1024, 256) bf16
    wot = io["wot"]                                    # (256, 1024) bf16
    bqc, bkc = io["bqc"], io["bkc"]                    # (128, 2) f32
    bvr = io["bvr"]                                    # (1, 256) bf16
    outp = io["outp"]                                  # (1024, 2048) f32
    EXP = mybir.ActivationFunctionType.Exp

    with (
        tc.tile_pool(name="const", bufs=1) as cw,
        tc.tile_pool(name="io", bufs=10) as iop,
        tc.tile_pool(name="big", bufs=1) as big,
        tc.tile_pool(name="work", bufs=3) as wk,
        tc.tile_pool(name="psA", bufs=2, space="PSUM") as psA,
        tc.tile_pool(name="psB", bufs=2, space="PSUM") as psB,
        tc.tile_pool(name="psC", bufs=2, space="PSUM") as psC,
    ):
        ones_sb = cw.tile([128, 128], BF16)
        nc.gpsimd.memset(ones_sb[:], 1.0)
        onesf = cw.tile([65, 64], F32)   # fp32 ones, row 64 used as K=1 lhsT
        nc.gpsimd.memset(onesf[:], 1.0)
        bq_sb = cw.tile([128, 2], F32)
        nc.sync.dma_start(bq_sb[:], bqc[:, :])
        bk_sb = cw.tile([128, 2], F32)
        nc.sync.dma_start(bk_sb[:], bkc[:, :])
        bv_sb = cw.tile([1, 256], BF16)
        nc.sync.dma_start(bv_sb[:], bvr[:, :])

        wq_sb = cw.tile([128, 8, 256], BF16)
        nc.sync.dma_start(wq_sb[:], wqt[:, :].rearrange("(k p) m -> p k m", p=128))
        wk_sb = cw.tile([128, 8, 256], BF16)
        nc.sync.dma_start(wk_sb[:], wkt[:, :].rearrange("(k p) m -> p k m", p=128))
        wv_sb = cw.tile([128, 8, 256], BF16)
        nc.sync.dma_start(wv_sb[:], wvt[:, :].rearrange("(k p) m -> p k m", p=128))
        wo_sb = cw.tile([128, 2, 1024], BF16)
        nc.sync.dma_start(wo_sb[:], wot[:, :].rearrange("(c p) m -> p c m", p=128))

        QT = big.tile([128, 2, S], BF16)   # [feat%128, feat//128, seq]
        # K^T kept as two half-zeroed copies so the scores matmul contracts
        # over the full 128 partitions (zeros kill the other head's Q rows);
        # K=64 matmuls read as "half-idle" to the PE activity monitor and the
        # clock gate kept re-throttling the whole attention phase.
        KTe = big.tile([128, 2, S], BF16)
        KTo = big.tile([128, 2, S], BF16)
        nc.gpsimd.memset(KTe[64:128, :, :], 0.0)
        nc.gpsimd.memset(KTo[0:64, :, :], 0.0)
        VA = big.tile([128, NHL, 16, 65], BF16)  # [key%128, head, key//128, dk|1]
        ON = big.tile([128, 2, S], BF16)   # normalized attn out, feature-major
        nc.gpsimd.memset(VA[:, :, :, 64:65], 1.0)

        # ---- Q/K projections, feature-major: dst[:, m, n] = W_local @ x^T ----
        # inputs stream as full 512KB rows (4KB/partition descriptors — small
        # per-partition runs were tanking HW-DGE efficiency)
        for xd, w_sb, b_sb, dst in (
            (qt, wq_sb, bq_sb, QT),
            (kt, wk_sb, bk_sb, None),
        ):
            rows = []
            for k in range(8):
                r = iop.tile([128, S], BF16, tag="xrow", name=f"xr{k}")
                nc.sync.dma_start(r[:], xd[k * 128 : (k + 1) * 128, :])
                rows.append(r)
            for n in range(NQ):
                pm = [
                    psA.tile([128, 512], F32, tag="proj", name=f"pm{m}")
                    for m in range(2)
                ]
                for k in range(8):
                    for m in range(2):
                        nc.tensor.matmul(
                            pm[m][:],
                            w_sb[:, k, m * 128 : (m + 1) * 128],
                            rows[k][:, n * 512 : (n + 1) * 512],
                            start=(k == 0),
                            stop=(k == 7),
                        )
                for m in range(2):
                    sl = slice(n * 512, (n + 1) * 512)
                    if dst is not None:
                        nc.vector.tensor_scalar_add(
                            dst[:, m, sl], pm[m][:], b_sb[:, m : m + 1]
                        )
                    else:
                        nc.vector.tensor_scalar_add(
                            KTe[0:64, m, sl], pm[m][0:64, :], b_sb[0:64, m : m + 1]
                        )
                        nc.vector.tensor_scalar_add(
                            KTo[64:128, m, sl], pm[m][64:128, :], b_sb[64:128, m : m + 1]
                        )

        # ---- V projection, natural layout, bias via K=1 ones matmul ----
        vrows = []
        for k in range(8):
            r = iop.tile([128, S], BF16, tag="xrow", name=f"vr{k}")
            nc.sync.dma_start(r[:], vt[k * 128 : (k + 1) * 128, :])
            vrows.append(r)
        for sp in range(8):
            pvps = psA.tile([128, 512], F32, tag="proj")
            for k in range(8):
                for half in range(2):
                    s = sp * 256 + half * 128
                    nc.tensor.matmul(
                        pvps[:, half * 256 : (half + 1) * 256],
                        vrows[k][:, s : s + 128],
                        wv_sb[:, k, :],
                        start=(k == 0 and half == 0),
                        stop=False,
                    )
            for half in range(2):
                nc.tensor.matmul(
                    pvps[:, half * 256 : (half + 1) * 256],
                    ones_sb[0:1, 0:128],
                    bv_sb[:],
                    start=False,
                    stop=(half == 1),
                )
            for half in range(2):
                s = sp * 2 + half
                nc.vector.tensor_copy(
                    VA[:, :, s, 0:64],
                    pvps[:, half * 256 : (half + 1) * 256].rearrange(
                        "p (h d) -> p h d", d=64
                    ),
                )

        # ---- attention + output projection, q-chunk-major for overlap ----
        # O-proj for chunk j is emitted after attention chunk j+1 so the PE
        # never waits on the normalize chain (reciprocal on DVE) of chunk j.
        def emit_oproj(j):
            for m in range(8):
                po = psA.tile([128, 512], F32, tag="proj", name="po")
                for c in range(2):
                    nc.tensor.matmul(
                        po[:],
                        wo_sb[:, c, m * 128 : (m + 1) * 128],
                        ON[:, c, j * 512 : (j + 1) * 512],
                        start=(c == 0),
                        stop=(c == 1),
                    )
                ot = wk.tile([128, 512], F32, tag="ot", name="ot")
                nc.any.tensor_copy(ot[:], po[:])
                nc.sync.dma_start(
                    outp[m * 128 : (m + 1) * 128, j * 512 : (j + 1) * 512], ot[:]
                )

        for j in range(NQ):
            nkc = 4 * (j + 1)
            us = []
            for h in range(NHL):
                p0 = 64 * (h % 2)
                ch = h // 2
                pvp = psC.tile([65, 512], F32, tag="pv", name="pvp")
                # exp batched over 2 key-chunks (one 2-bank PSUM tile) to
                # amortize the ~350ns fixed cost per ACTIVATE
                for kp in range(nkc // 2):
                    scp = psB.tile([128, 1024], F32, tag="sc", name="scp")
                    ktz = KTe if h % 2 == 0 else KTo
                    for half in range(2):
                        kc = 2 * kp + half
                        nc.tensor.matmul(
                            scp[:, half * 512 : (half + 1) * 512],
                            ktz[:, ch, kc * 128 : (kc + 1) * 128],
                            QT[:, ch, j * 512 : (j + 1) * 512],
                            start=True,
                            stop=True,
                        )
                    et = wk.tile([128, 1024], BF16, tag="exp", name="et")
                    nc.scalar.activation(et[:], scp[:], EXP, scale=float(DK) ** -0.5)
                    for half in range(2):
                        kc = 2 * kp + half
                        if kc >= 4 * j:
                            t = kc - 4 * j
                            nc.gpsimd.affine_select(
                                out=et[:, half * 512 : (half + 1) * 512],
                                in_=et[:, half * 512 : (half + 1) * 512],
                                compare_op=mybir.AluOpType.is_ge,
                                fill=0.0,
                                base=-128 * t,
                                pattern=[[1, 512]],
                                channel_multiplier=-1,
                            )
                    for half in range(2):
                        kc = 2 * kp + half
                        nc.tensor.matmul(
                            pvp[:],
                            VA[:, h, kc, :],
                            et[:, half * 512 : (half + 1) * 512],
                            start=(kc == 0),
                            stop=(kc == nkc - 1),
                        )
                # copy the PV accumulator to SBUF immediately — freeing the
                # pv PSUM slot fast keeps the PE from stalling (and HAM from
                # re-throttling the clock); normalization is deferred below.
                u = wk.tile([65, 512], F32, tag="u", name="u", bufs=6)
                nc.any.tensor_copy(u[:], pvp[:])
                us.append(u)
            # O-projection of the previous chunk keeps the PE fed while this
            # chunk's reciprocals run on the DVE
            if j > 0:
                emit_oproj(j - 1)
            # normalize: 1/Z in place on the SBUF Z row (all reciprocals at
            # the chunk boundary so they never delay the PSUM-freeing copies
            # in the DVE queue), broadcast across 64 partitions with a K=1
            # fp32 matmul, scale into ON
            for h in range(NHL):
                u = us[h]
                nc.vector.reciprocal(u[64:65, :], u[64:65, :])
                rbps = psA.tile([64, 512], F32, tag="proj", name="rbps")
                nc.tensor.matmul(
                    rbps[:], onesf[64:65, :], u[64:65, :], start=True, stop=True
                )
                if h % 2 == 0:
                    nc.vector.tensor_mul(
                        ON[0:64, h // 2, j * 512 : (j + 1) * 512], u[0:64, :], rbps[:]
                    )
                else:
                    nt = wk.tile([64, 512], BF16, tag="nt", name="nt")
                    nc.vector.tensor_mul(nt[:], u[0:64, :], rbps[:])
                    nc.sync.dma_start(
                        ON[64:128, h // 2, j * 512 : (j + 1) * 512], nt[:]
                    )
        emit_oproj(NQ - 1)


def build_nc():
    nc = bacc.Bacc("TRN2", target_bir_lowering=False, debug=False, num_devices=8)
    io = {}
    for name, shape, dt in (
        ("qt", (D, S), BF16),
        ("kt", (D, S), BF16),
        ("vt", (D, S), BF16),
        ("wqt", (D, DL), BF16),
        ("wkt", (D, DL), BF16),
        ("wvt", (D, DL), BF16),
        ("wot", (DL, D), BF16),
        ("bqc", (128, 2), F32),
        ("bkc", (128, 2), F32),
        ("bvr", (1, DL), BF16),
    ):
        io[name] = nc.dram_tensor(name, shape, dt, kind="ExternalInput")
    io["outp"] = nc.dram_tensor("outp", (D, S), F32, kind="ExternalOutput")
    with tile.TileContext(nc) as tc:
        _emit(tc, io)
    nc.compile()
    return nc


_NC = None


def _get_nc():
    global _NC
    if _NC is None:
        _NC = build_nc()
    return _NC


def make_in_maps(q, k, v, Wq, bq, Wk, bk, Wv, bv, Wo):
    def cb(x):  # contiguous bf16
        return np.ascontiguousarray(x).astype(NPBF16)

    cf = np.ascontiguousarray
    in_maps = []
    for core in range(8):
        b, g = divmod(core, 4)
        sl = slice(DL * g, DL * (g + 1))
        in_maps.append(
            {
                "qt": cb(q[b].T),
                "kt": cb(k[b].T),
                "vt": cb(v[b].T),
                "wqt": cb(Wq[sl, :].T),
                "wkt": cb(Wk[sl, :].T),
                "wvt": cb(Wv[sl, :].T),
                "wot": cb(Wo[:, sl].T),
                "bqc": cf(bq[sl].reshape(2, 128).T),
                "bkc": cf(bk[sl].reshape(2, 128).T),
                "bvr": cb(bv[sl].reshape(1, DL)),
            }
        )
    return in_maps


def gather_output(results, bo):
    out = np.empty((B, S, D), np.float32)
    for b in range(B):
        acc = results[4 * b]["outp"].astype(np.float32)
        for g in range(1, 4):
            acc = acc + results[4 * b + g]["outp"]
        out[b] = acc.T + bo
    return out


def _np_fallback(q, k, v, mask, Wq, bq, Wk, bk, Wv, bv, Wo, bo):
    # generic-mask reference path; only used if the mask is not causal
    out = np.empty((B, S, D), np.float32)
    m = np.broadcast_to(mask, (B, 1, S, S))
    for b in range(B):
        Q = (q[b] @ Wq.T + bq).reshape(S, H, DK).transpose(1, 0, 2)
        K = (k[b] @ Wk.T + bk).reshape(S, H, DK).transpose(1, 0, 2)
        V = (v[b] @ Wv.T + bv).reshape(S, H, DK).transpose(1, 0, 2)
        o = np.empty((H, S, DK), np.float32)
        for hh in range(H):
            s = (Q[hh] @ K[hh].T) * (DK**-0.5)
            s = np.where(m[b, 0] == 0, -np.inf, s)
            s = s - s.max(axis=-1, keepdims=True)
            e = np.exp(s)
            o[hh] = (e / e.sum(axis=-1, keepdims=True)) @ V[hh]
        out[b] = o.transpose(1, 0, 2).reshape(S, D) @ Wo.T + bo
    return out


def kernel(q, k, v, mask, Wq, bq, Wk, bk, Wv, bv, Wo, bo):
    f32 = np.float32
    q, k, v = (np.asarray(x, f32) for x in (q, k, v))
    Wq, bq, Wk, bk = (np.asarray(x, f32) for x in (Wq, bq, Wk, bk))
    Wv, bv, Wo, bo = (np.asarray(x, f32) for x in (Wv, bv, Wo, bo))
    mask = np.asarray(mask)

    if not np.array_equal(
        np.broadcast_to(mask, (1, 1, S, S))[0, 0] != 0,
        np.tril(np.ones((S, S), bool)),
    ):
        return _np_fallback(q, k, v, mask, Wq, bq, Wk, bk, Wv, bv, Wo, bo)

    nc = _get_nc()
    in_maps = make_in_maps(q, k, v, Wq, bq, Wk, bk, Wv, bv, Wo)
    res = run_bass_kernel_spmd(nc, in_maps, list(range(8)))
    return gather_output(res.results, bo)
